# revision 2
# baseline (speedup 1.0000x reference)
# EpLSTMCell Trainium2 kernel: tensor-parallel over the gate dimension with a
# per-step AllGather of the hidden state, optionally combined with batch-group
# parallelism (NGRP batch groups x GSZ-way gate split, NGRP*GSZ = 8 cores).
#
# Math (per step t):
#   g  = x_t @ Wx.T + h_{t-1} @ Wh.T + (bx + bh)      # gates, order I,F,G,O,R
#   ft = sigmoid(gF); gt = tanh(gG); ot = sigmoid(gO); rt = sigmoid(gR)
#   it = 1 - ft  (tied -> the I-gate slice of Wx/Wh/b is mathematically unused)
#   c  = ft*c + (1-ft)*gt + rt*tanh(m_t) = gt + ft*(c-gt) + rt*tanh(m_t)
#   h  = ot*tanh(c);  out[t] = h
#
# Core j (group g = j//GSZ, slot q = j%GSZ) handles batch rows
# [BL*g, BL*(g+1)) and Dh slice [SLC*q, SLC*(q+1)) of gates F,G,O,R.
# Per step it accumulates its [BL, 4*SLC] gate block in PSUM (bias rank-1
# matmul + 8 x-K-tiles + 8 h-K-tiles, bf16 in / fp32 accumulate), applies the
# cell update, transposes its h slice on the PE and AllGathers h^T within its
# group so every core has the full [1024, BL] h^T for the next step.
#
# All weight reshapes/transposes/casts and tanh(memories) are host-side numpy.

import os

import numpy as np
import ml_dtypes

T, B, DX, DH = 128, 128, 1024, 1024
NC = 8
KT = DX // 128       # 8 contraction tiles
NG = 4               # gates kept: F, G, O, R (I dropped: it = 1-ft)

_NC_CACHE = {}


def _geom(ngrp):
    gsz = NC // ngrp          # cores per gate-split group
    bl = B // ngrp            # batch rows per group
    slc = DH // gsz           # Dh slice per core
    gw = NG * slc             # gate columns per core
    return gsz, bl, slc, gw


def _build_nc(n_steps=T, repeat=1, use_ag=True, ngrp=1):
    import concourse.mybir as mybir
    import concourse.tile as tile
    from concourse import bacc
    from concourse.masks import make_identity

    gsz, bl, slc, gw = _geom(ngrp)
    ntp = slc // 128          # PE sub-transposes per step
    f32 = mybir.dt.float32
    bf16 = mybir.dt.bfloat16
    Sig = mybir.ActivationFunctionType.Sigmoid
    Tanh = mybir.ActivationFunctionType.Tanh
    Copy = mybir.ActivationFunctionType.Copy

    nc = bacc.Bacc(
        "TRN2",
        target_bir_lowering=False,
        debug=False,
        enable_asserts=False,
        num_devices=NC,
    )

    xT = nc.dram_tensor("xT", [n_steps, 128, KT, bl], bf16, kind="ExternalInput").ap()
    h0T = nc.dram_tensor("h0T", [128, KT, bl], bf16, kind="ExternalInput").ap()
    wxT = nc.dram_tensor("wxT", [128, KT, gw], bf16, kind="ExternalInput").ap()
    whT = nc.dram_tensor("whT", [128, KT, gw], bf16, kind="ExternalInput").ap()
    biasr = nc.dram_tensor("biasr", [1, gw], bf16, kind="ExternalInput").ap()
    mt = nc.dram_tensor("mt", [n_steps, bl, slc], f32, kind="ExternalInput").ap()
    c0 = nc.dram_tensor("c0", [bl, slc], f32, kind="ExternalInput").ap()
    out = nc.dram_tensor("out", [n_steps, bl, slc], f32, kind="ExternalOutput").ap()

    groups = [[g * gsz + i for i in range(gsz)] for g in range(ngrp)]
    gath_space = "Shared" if gsz > 4 else "Local"

    def mm_chunks(total, maxn=512):
        o = 0
        while o < total:
            n = min(maxn, total - o)
            yield o, n
            o += n

    with tile.TileContext(nc) as tc:
        with (
            tc.tile_pool(name="const", bufs=1) as constp,
            tc.tile_pool(name="xs", bufs=3) as xsp,
            tc.tile_pool(name="hs", bufs=2) as hsp,
            tc.tile_pool(name="ms", bufs=3) as msp,
            tc.tile_pool(name="cell", bufs=2) as cellp,
            tc.tile_pool(name="act", bufs=2) as actp,
            tc.tile_pool(name="ps", bufs=2, space="PSUM") as psp,
            tc.tile_pool(name="pst", bufs=2, space="PSUM") as pstp,
            tc.tile_pool(name="drin", bufs=2, space="DRAM") as drinp,
            tc.tile_pool(name="drout", bufs=2, space="DRAM") as droutp,
        ):
            wx_sb = constp.tile([128, KT, gw], bf16)
            nc.sync.dma_start(wx_sb[:], wxT[:])
            wh_sb = constp.tile([128, KT, gw], bf16)
            nc.sync.dma_start(wh_sb[:], whT[:])
            bias_sb = constp.tile([1, gw], bf16)
            nc.sync.dma_start(bias_sb[:], biasr[:])
            ones_sb = constp.tile([1, bl], bf16)
            nc.gpsimd.memset(ones_sb[:], 1.0)
            ident_sb = constp.tile([bl, bl], f32)
            make_identity(nc, ident_sb[:])

            c_prev = cellp.tile([bl, slc], f32, tag="c", name="c_init")
            nc.sync.dma_start(c_prev[:], c0[:])
            h_sb = hsp.tile([128, KT, bl], bf16, tag="h", name="h_init")
            nc.sync.dma_start(h_sb[:], h0T[:])

            half = gw // 2

            def xpart(t):
                # open gate accumulation for step t: bias + x-projection.
                # Two PSUM tiles (F,G | O,R) in separate banks so the F,G
                # activations can read while O,R matmuls still stream.
                g_a = psp.tile([bl, half], f32, tag="ga", name=f"ga_{t}")
                g_b = psp.tile([bl, half], f32, tag="gb", name=f"gb_{t}")
                for g_ps, base in ((g_a, 0), (g_b, half)):
                    for o, n in mm_chunks(half):
                        nc.tensor.matmul(
                            g_ps[:, o : o + n], ones_sb[:],
                            bias_sb[:, base + o : base + o + n],
                            start=True, stop=False,
                        )
                x_sb = xsp.tile([128, KT, bl], bf16, tag="x", name=f"x_{t}")
                nc.sync.dma_start(x_sb[:], xT[t])
                for k in range(KT):
                    for g_ps, base in ((g_a, 0), (g_b, half)):
                        for o, n in mm_chunks(half):
                            nc.tensor.matmul(
                                g_ps[:, o : o + n], x_sb[:, k, :],
                                wx_sb[:, k, base + o : base + o + n],
                                start=False, stop=False,
                            )
                return g_a, g_b

            g_cur = xpart(0)
            total_steps = n_steps * repeat
            for tt in range(total_steps):
                t = tt % n_steps
                # recurrent part: g += h^T K-tiles (stationary) x Wh^T (moving).
                # F,G half first (own PSUM bank) so its K-loop finishes early
                # and the activation/cell chain overlaps the O,R streaming.
                g_a, g_b = g_cur
                for g_ps, base in ((g_a, 0), (g_b, half)):
                    for k in range(KT):
                        for o, n in mm_chunks(half):
                            nc.tensor.matmul(
                                g_ps[:, o : o + n], h_sb[:, k, :],
                                wh_sb[:, k, base + o : base + o + n], start=False,
                                stop=(k == KT - 1 and o + n == half),
                            )

                sa = actp.tile([bl, 2 * slc], f32, tag="sa", name=f"sa_{t}")
                nc.scalar.activation(sa[:], g_a[:, 0 : 2 * slc], Sig)
                ft = sa[:, 0:slc]
                gt = actp.tile([bl, slc], f32, tag="gt", name=f"gt_{t}")
                nc.scalar.activation(gt[:], sa[:, slc : 2 * slc], Copy,
                                     bias=-1.0, scale=2.0)
                sb_ = actp.tile([bl, 2 * slc], f32, tag="sb", name=f"sb_{t}")
                nc.scalar.activation(sb_[:], g_b[:, 0 : 2 * slc], Sig)
                ot = sb_[:, 0:slc]
                rt = sb_[:, slc : 2 * slc]

                m_sb = msp.tile([bl, slc], f32, tag="m", name=f"m_{t}")
                nc.scalar.dma_start(m_sb[:], mt[t])

                s = actp.tile([bl, slc], f32, tag="s", name=f"s_{t}")
                nc.vector.tensor_sub(s[:], c_prev[:], gt[:])
                p = actp.tile([bl, slc], f32, tag="p", name=f"p_{t}")
                nc.vector.tensor_mul(p[:], ft, s[:])
                q = actp.tile([bl, slc], f32, tag="q", name=f"q_{t}")
                nc.vector.tensor_mul(q[:], rt, m_sb[:])
                u = actp.tile([bl, slc], f32, tag="u", name=f"u_{t}")
                nc.vector.tensor_add(u[:], gt[:], p[:])
                c_new = cellp.tile([bl, slc], f32, tag="c", name=f"c_{t}")
                nc.vector.tensor_add(c_new[:], u[:], q[:])
                th = actp.tile([bl, slc], f32, tag="th", name=f"th_{t}")
                nc.scalar.activation(th[:], c_new[:], Tanh)
                h_f = cellp.tile([bl, slc], f32, tag="hf", name=f"hf_{t}")
                nc.vector.tensor_mul(h_f[:], ot, th[:])
                nc.scalar.dma_start(out[t], h_f[:])
                c_prev = c_new

                if tt < total_steps - 1:
                    # h slice -> h^T (PE), PSUM -> SBUF bf16, -> DRAM bounce
                    tp = pstp.tile([128, ntp, bl], f32, tag="tp", name=f"tp_{t}")
                    for i in range(ntp):
                        nc.tensor.transpose(
                            tp[:, i, :], h_f[:, i * 128 : (i + 1) * 128], ident_sb[:]
                        )
                    hT_sb = actp.tile([128, ntp, bl], bf16, tag="hT", name=f"hT_{t}")
                    nc.scalar.activation(hT_sb[:], tp[:], Copy)
                    bounce = drinp.tile([slc, bl], bf16, tag="bounce", name=f"bounce_{t}")
                    nc.sync.dma_start(
                        bounce[:].rearrange("(i p) b -> p i b", p=128), hT_sb[:]
                    )
                    if use_ag:
                        gathered = droutp.tile(
                            [DH, bl], bf16, addr_space=gath_space, tag="gath",
                            name=f"gath_{t}",
                        )
                        nc.gpsimd.collective_compute(
                            "AllGather",
                            mybir.AluOpType.bypass,
                            replica_groups=groups,
                            ins=[bounce.opt()],
                            outs=[gathered.opt()],
                        )
                        h_sb = hsp.tile([128, KT, bl], bf16, tag="h", name=f"h_{t}")
                        gath_r = gathered[:].rearrange("(k p) b -> p k b", p=128)
                        # split so the first h-matmuls can start before the
                        # whole gathered tensor has landed in SBUF
                        nc.sync.dma_start(h_sb[:, 0:1, :], gath_r[:, 0:1, :])
                        nc.sync.dma_start(h_sb[:, 1:3, :], gath_r[:, 1:3, :])
                        nc.sync.dma_start(h_sb[:, 3:KT, :], gath_r[:, 3:KT, :])
                    # (no-AG timing variant: keep stale h_sb; math is wrong but
                    # the instruction mix minus the collective chain is kept)
                    g_cur = xpart((tt + 1) % n_steps)

    nc.compile()
    return nc


def _get_nc(n_steps=T, repeat=1, use_ag=True, ngrp=1):
    key = (n_steps, repeat, use_ag, ngrp)
    if key not in _NC_CACHE:
        _NC_CACHE[key] = _build_nc(n_steps, repeat, use_ag, ngrp)
    return _NC_CACHE[key]


def _prep_in_maps(inputs, n_steps=T, ngrp=1):
    gsz, bl, slc, gw = _geom(ngrp)
    bf16 = ml_dtypes.bfloat16
    x = np.asarray(inputs["inputs"], np.float32)[:n_steps]
    m = np.asarray(inputs["memories"], np.float32)[:n_steps]
    h0 = np.asarray(inputs["h0"], np.float32)
    c0 = np.asarray(inputs["c0"], np.float32)
    Wx = np.asarray(inputs["Wx"], np.float32)
    bx = np.asarray(inputs["bx"], np.float32)
    Wh = np.asarray(inputs["Wh"], np.float32)
    bh = np.asarray(inputs["bh"], np.float32)
    bias = bx + bh

    # per batch-group: xT[t, p, k, b] = x[t, bl*g + b, 128k+p]
    xTg, h0Tg = [], []
    for g in range(ngrp):
        xg = x[:, g * bl : (g + 1) * bl, :]
        xTg.append(
            np.ascontiguousarray(
                xg.reshape(n_steps, bl, KT, 128).transpose(0, 3, 2, 1)
            ).astype(bf16)
        )
        h0g = h0[g * bl : (g + 1) * bl]
        h0Tg.append(
            np.ascontiguousarray(h0g.reshape(bl, KT, 128).transpose(2, 1, 0)).astype(bf16)
        )

    wxTq, whTq, biasq = [], [], []
    for q in range(gsz):
        rows = np.concatenate(
            [np.arange(gb * DH + q * slc, gb * DH + (q + 1) * slc) for gb in (1, 2, 3, 4)]
        )
        wxq = Wx[rows].copy()
        whq = Wh[rows].copy()
        bq = bias[rows].copy()
        # G block is rows slc:2*slc of the [F,G,O,R] concat (gb order 1,2,3,4
        # = F,G,O,R): double it so tanh(G) = 2*sigmoid(2G)-1 comes out of the
        # batched [F,G] sigmoid plus one affine Copy.
        wxq[slc : 2 * slc] *= 2.0
        whq[slc : 2 * slc] *= 2.0
        bq[slc : 2 * slc] *= 2.0
        wxTq.append(
            np.ascontiguousarray(
                wxq.T.reshape(KT, 128, gw).transpose(1, 0, 2)
            ).astype(bf16)
        )
        whTq.append(
            np.ascontiguousarray(
                whq.T.reshape(KT, 128, gw).transpose(1, 0, 2)
            ).astype(bf16)
        )
        biasq.append(np.ascontiguousarray(bq.reshape(1, gw)).astype(bf16))

    in_maps = []
    for j in range(NC):
        g, q = j // gsz, j % gsz
        mtj = np.tanh(
            m[:, g * bl : (g + 1) * bl, q * slc : (q + 1) * slc]
        ).astype(np.float32)
        c0j = np.ascontiguousarray(
            c0[g * bl : (g + 1) * bl, q * slc : (q + 1) * slc]
        ).astype(np.float32)
        in_maps.append(
            {
                "xT": xTg[g],
                "h0T": h0Tg[g],
                "wxT": wxTq[q],
                "whT": whTq[q],
                "biasr": biasq[q],
                "mt": mtj,
                "c0": c0j,
            }
        )
    return in_maps


def _assemble(results, ngrp=1):
    gsz, bl, slc, gw = _geom(ngrp)
    n_steps = results[0]["out"].shape[0]
    full = np.zeros((n_steps, B, DH), np.float32)
    for j in range(NC):
        g, q = j // gsz, j % gsz
        full[:, g * bl : (g + 1) * bl, q * slc : (q + 1) * slc] = results[j]["out"]
    return full


NGRP_DEFAULT = 1


def _run(inputs, n_steps=T, trace=False, ngrp=NGRP_DEFAULT):
    from concourse import bass_utils

    nc = _get_nc(n_steps, 1, True, ngrp)
    in_maps = _prep_in_maps(inputs, n_steps, ngrp)
    res = bass_utils.run_bass_kernel_spmd(
        nc, in_maps, core_ids=list(range(NC)), trace=trace
    )
    return _assemble(res.results, ngrp), res


def kernel(**inputs) -> np.ndarray:
    full, _ = _run(inputs, T, trace=bool(os.environ.get("EPLSTM_TRACE")))
    return full

